# revision 14
# baseline (speedup 1.0000x reference)
"""Trainium2 Bass kernel for the bipartite GNN message-passing encoder.

Math (see reference.py):
  A_r = (adj == r), r = 1..5
  An_r = diag(1/sqrt(Nu)) A_r diag(1/sqrt(Nv))
  Hu = relu(sum_r An_r @ W_items_r^T)   [NU, M]
  Hv = relu(sum_r An_r^T @ W_users_r^T) [NI, M]
  U  = relu(Hu @ dense_W^T + relu(u_sideFeat @ u_W1^T + u_b1) @ u_W2^T)
  V  = relu(Hv @ dense_W^T + relu(v_sideFeat @ v_W1^T + v_b1) @ v_W2^T)

Sharding: symmetric 1D. Core c owns users U_c = [500c, 500c+500) and items
I_c = [500c, 500c+500). The host hands each core TWO adjacency views in
int8: adjR = adj[U_c, :] (full rows) and adjCT = adj[:, I_c]^T (full
columns, pre-transposed). Row degrees for U_c and column degrees for I_c
are therefore LOCAL - no degree collectives - so the pass-1 mask-matmul
streams start within ~8us of launch. Each stream produces a partial over
the full opposite side (HvT partial [M, NI] from my users; HuT partial
[M, NU] from my items), laid out in DRAM blocked by destination core
[8, M, 500] and combined with a single bf16 ReduceScatter each. Pass 2
is then fully local.

A 4-byte dummy AllReduce is triggered as the first instruction so the
collectives init barrier (which waits for the slowest core's trigger)
overlaps the local compute instead of delaying the first real
collective.

Engine budget: PE runs 640 back-to-back [<=125c x 128 x 500] bf16
matmuls (~165us at the 81% GPIO clock limit; no transposes - everything
arrives host-pre-transposed). DVE builds the 80 masks (dual-op is_equal
x per-partition degree factor) and the degree factors (fused
nz+rowsum via accum_out, then max+pow(-0.5)). ACT evacuates PSUM and
does pass-2 activations. DMA issue serializes per HW DGE queue at
~130GB/s, so the inputs ride TWO queues: sync gets adjR interleaved
with the per-kt weight chunks (first matmul operands land first);
scalar gets adjCT/wiH/side tensors (needed only from ~45us on).
"""

import sys

import numpy as np

if "/opt/trn_rl_repo" not in sys.path:
    sys.path.insert(0, "/opt/trn_rl_repo")

import concourse.bacc as bacc  # noqa: E402
import concourse.mybir as mybir  # noqa: E402
import concourse.tile as tile  # noqa: E402

FP = mybir.dt.float32
BF = mybir.dt.bfloat16
I8 = mybir.dt.int8

NU = NI = 4000
R = 5
M = 256
OUT = 75
SIDE = 64
FDIM = 128

NCORES = 8
BU = NU // NCORES  # 500 users per core
BI = NI // NCORES  # 500 items per core

AF = mybir.ActivationFunctionType
ALU = mybir.AluOpType

ALL_GROUP = [list(range(NCORES))]
PAIR_GROUPS = [[2 * a, 2 * a + 1] for a in range(NCORES // 2)]

PT = [(t * 125, 125) for t in range(4)]  # 4 partition tiles over 500
WK = R * M  # 1280 packed weight columns per kt chunk
# smallpack column layout
SP_DW = 0  # [128, 2x75] dense_W^T halves
SP_UW1 = 150  # [128, 64]
SP_VW1 = 214  # [128, 64]
SP_UW2 = 278  # [64, 75]
SP_VW2 = 353  # [64, 75]
SP_COLS = 428


def build_program():
    from contextlib import ExitStack

    nc = bacc.Bacc("TRN2", target_bir_lowering=False, debug=False, num_devices=NCORES)

    # ---- I/O ---- (all host-sliced / packed / pre-transposed)
    adjR = nc.dram_tensor("adjR", [BU, NI], I8, kind="ExternalInput")
    adjCT = nc.dram_tensor("adjCT", [BI, NU], I8, kind="ExternalInput")
    # packed msg_W: [4kt][125, R*M] with col (r*M + m)
    wuH = nc.dram_tensor("wuH", [4, 125, WK], BF, kind="ExternalInput")
    wiH = nc.dram_tensor("wiH", [4, 125, WK], BF, kind="ExternalInput")
    ufT = nc.dram_tensor("ufT", [FDIM, BU], BF, kind="ExternalInput")
    vfT = nc.dram_tensor("vfT", [FDIM, BI], BF, kind="ExternalInput")
    smallpack = nc.dram_tensor("smallpack", [128, SP_COLS], BF, kind="ExternalInput")
    ub1 = nc.dram_tensor("ub1", [SIDE, 1], FP, kind="ExternalInput")
    vb1 = nc.dram_tensor("vb1", [SIDE, 1], FP, kind="ExternalInput")
    u_out = nc.dram_tensor("u_out", [BU, OUT], FP, kind="ExternalOutput")
    v_out = nc.dram_tensor("v_out", [BI, OUT], FP, kind="ExternalOutput")

    with tile.TileContext(nc) as tc, ExitStack() as ctx:
        res = ctx.enter_context(tc.tile_pool(name="res", bufs=1))
        scr = ctx.enter_context(tc.tile_pool(name="scr", bufs=2))
        dram = ctx.enter_context(tc.tile_pool(name="dram", bufs=1, space="DRAM"))

        # ---- dummy collective: absorbs the init barrier during compute ----
        dummy_src = res.tile([1, 8], FP, tag="dummy_src")
        nc.gpsimd.memset(dummy_src[:], 0.0)
        dram_dmy = dram.tile([1, 8], FP, tag="dram_dmy")
        dram_dmy_o = dram.tile([1, 8], FP, tag="dram_dmy_o")
        nc.scalar.dma_start(out=dram_dmy[:, :], in_=dummy_src[:, :])
        nc.gpsimd.collective_compute(
            "AllReduce", ALU.add, replica_groups=PAIR_GROUPS,
            ins=[dram_dmy.opt()], outs=[dram_dmy_o.opt()],
        )

        # ---- input DMAs: sync = adjR + wu chunks interleaved (kt order);
        # i8 on the wire, ACT converts each tile to bf16 (masks are 2x
        # faster on DVE from bf16, and ACT is otherwise idle early)
        adjR_i8, adjR_t, wu_sb = [], [], []
        for kt, (s, p) in enumerate(PT):
            ai = scr.tile([128, NI], I8, tag="ai8", bufs=2, name="ai")
            nc.sync.dma_start(out=ai[:p, :], in_=adjR[s : s + p, :])
            adjR_i8.append(ai)
            adjR_t.append(res.tile([128, NI], BF, tag=f"aR{kt}", name="ab"))
            wt = res.tile([125, WK], BF, tag=f"wu{kt}", name="wt")
            nc.sync.dma_start(out=wt[:, :], in_=wuH[kt, :, :])
            wu_sb.append(wt)
        adjCT_i8 = [
            scr.tile([128, NU], I8, tag="ci8", bufs=3, name="ci") for _ in range(4)
        ]

        # ---- scalar-queue tensors: tiles allocated now, DMAs emitted inside
        # pass-1 preps (interleaved with the degree sqrts so neither blocks)
        adjCT_t = [
            res.tile([128, NU], BF, tag=f"aC{kt}", name="ac") for kt in range(4)
        ]
        wi_sb = [res.tile([125, WK], BF, tag=f"wi{kt}", name="wt") for kt in range(4)]
        ufT_sb = res.tile([128, BU], BF, tag="ufT_sb")
        vfT_sb = res.tile([128, BI], BF, tag="vfT_sb")
        sp_sb = res.tile([128, SP_COLS], BF, tag="sp_sb")
        ub1_t = res.tile([SIDE, 1], FP, tag="ub1_t")
        vb1_t = res.tile([SIDE, 1], FP, tag="vb1_t")

        def wsl(w_sb, r, kt, mh):  # packed lhsT slice [125, 128]
            c = r * M + mh * 128
            return w_sb[kt][:125, c : c + 128]

        # ---- local degree factors (sqrt on ACT), emitted lazily ----
        a_fac = [None] * 4
        b_fac = [None] * 4

        def emit_deg(adj_t, fac, kt, nm):
            p = PT[kt][1]
            nz = scr.tile([128, NI], BF, tag="nz", bufs=2, name="nz")
            nc.vector.tensor_scalar(
                out=nz[:p, :], in0=adj_t[kt][:p, :], scalar1=1.0, scalar2=None,
                op0=ALU.min,
            )
            dg = scr.tile([128, 1], FP, tag="dg", bufs=2, name="dg")
            nc.vector.tensor_reduce(
                out=dg[:p, :], in_=nz[:p, :], axis=mybir.AxisListType.X, op=ALU.add,
            )
            m1 = scr.tile([128, 1], FP, tag="m1", bufs=2, name="m1")
            nc.vector.tensor_scalar(
                out=m1[:p, :], in0=dg[:p, :], scalar1=1.0, scalar2=None, op0=ALU.max,
            )
            sq = scr.tile([128, 1], FP, tag="sq", bufs=2, name="sq")
            nc.scalar.sqrt(out=sq[:p, :], in_=m1[:p, :])
            fc = res.tile([128, 1], FP, tag=f"{nm}fac{kt}", name="fc")
            nc.vector.reciprocal(out=fc[:p, :], in_=sq[:p, :])
            fac[kt] = fc

        ps_mm = tc.alloc_tile_pool(name="ps_mm", bufs=1, space="PSUM")

        # DRAM partial buffers, blocked by destination core [8, M, 500]
        dram_hv = dram.tile([NCORES, M, BI], BF, tag="dram_hv")
        dram_hu = dram.tile([NCORES, M, BU], BF, tag="dram_hu")
        dram_hv_red = dram.tile([M, BI], BF, tag="dram_hv_red")
        dram_hu_red = dram.tile([M, BU], BF, tag="dram_hu_red")

        # ---- pass 1: one side = 2 halves x (4kt x 5r masks -> 8-bank matmul) ----
        def pass1(adj_t, fac, w_sb, w_blk, dram_part, prep):
            # partial H^T[m, col] = sum_r sum_p (fac_p * mask_r[p, col]) * W[r][m, p]
            for h in range(2):
                P = [
                    [
                        ps_mm.tile([128, w_blk], FP, tag=f"p{mh}{cc}", name="P")
                        for cc in range(4)
                    ]
                    for mh in range(2)
                ]
                for kt, (s, p) in enumerate(PT):
                    if prep is not None:
                        prep(h, kt)
                    for r in range(R):
                        msk = scr.tile(
                            [128, 4 * w_blk], BF, tag="mask", bufs=3, name="msk"
                        )
                        nc.vector.tensor_scalar(
                            out=msk[:p, :],
                            in0=adj_t[kt][:p, h * 4 * w_blk : (h + 1) * 4 * w_blk],
                            scalar1=float(r + 1), scalar2=fac[kt][:p, :],
                            op0=ALU.is_equal, op1=ALU.mult,
                        )
                        first = kt == 0 and r == 0
                        last = kt == 3 and r == R - 1
                        for mh in range(2):
                            for cc in range(4):
                                nc.tensor.matmul(
                                    P[mh][cc][:, :],
                                    lhsT=wsl(w_sb, r, kt, mh),
                                    rhs=msk[:p, cc * w_blk : (cc + 1) * w_blk],
                                    start=first, stop=last,
                                )
                # evacuate in matmul emission order so the next half's first
                # matmul only waits on its own bank
                for mh in range(2):
                    for cc in range(4):
                        ev = scr.tile([128, w_blk], BF, tag="ev", bufs=4, name="ev")
                        nc.scalar.copy(out=ev[:, :], in_=P[mh][cc][:, :])
                        nc.sync.dma_start(
                            out=dram_part[h * 4 + cc, mh * 128 : (mh + 1) * 128, :],
                            in_=ev[:, :],
                        )

        def item_prep(h, kt):
            s, p = PT[kt]
            if h == 0:
                # bf16 convert + aR degree chain, then this kt's adjCT load
                # right behind the sqrt in the scalar FIFO
                nc.scalar.copy(out=adjR_t[kt][:p, :], in_=adjR_i8[kt][:p, :])
                emit_deg(adjR_t, a_fac, kt, "a")
                nc.scalar.dma_start(
                    out=adjCT_i8[kt][:p, :], in_=adjCT[s : s + p, :]
                )
            else:
                nc.scalar.copy(out=adjCT_t[kt][:p, :], in_=adjCT_i8[kt][:p, :])
                emit_deg(adjCT_t, b_fac, kt, "b")
                if kt == 0:
                    for k2 in range(4):
                        nc.scalar.dma_start(out=wi_sb[k2][:, :], in_=wiH[k2, :, :])
                elif kt == 1:
                    nc.scalar.dma_start(out=ufT_sb[:, :], in_=ufT[:, :])
                    nc.scalar.dma_start(out=vfT_sb[:, :], in_=vfT[:, :])
                elif kt == 2:
                    nc.scalar.dma_start(out=sp_sb[:, :], in_=smallpack[:, :])
                    nc.scalar.dma_start(out=ub1_t[:, :], in_=ub1[:, :])
                    nc.scalar.dma_start(out=vb1_t[:, :], in_=vb1[:, :])

        pass1(adjR_t, a_fac, wu_sb, BI, dram_hv, item_prep)
        nc.gpsimd.collective_compute(
            "ReduceScatter", ALU.add, replica_groups=ALL_GROUP,
            ins=[dram_hv.opt()], outs=[dram_hv_red.opt()],
        )
        pass1(adjCT_t, b_fac, wi_sb, BU, dram_hu, None)
        nc.gpsimd.collective_compute(
            "ReduceScatter", ALU.add, replica_groups=ALL_GROUP,
            ins=[dram_hu.opt()], outs=[dram_hu_red.opt()],
        )

        ps_mm.release()
        ps_p2 = ctx.enter_context(tc.tile_pool(name="ps_p2", bufs=2, space="PSUM"))

        # ---- side-feature heads (PE is free once pass 1 drains) ----
        def side_head(w1c, bia, sft, n, nm):
            fT = res.tile([SIDE, n], BF, tag=f"fT_{nm}", name="fT")
            pf = ps_p2.tile([SIDE, n], FP, tag="pf", name="pf")
            nc.tensor.matmul(
                pf[:, :], lhsT=sp_sb[:FDIM, w1c : w1c + SIDE], rhs=sft[:FDIM, :],
                start=True, stop=True,
            )
            nc.scalar.activation(
                out=fT[:, :], in_=pf[:, :], func=AF.Relu, bias=bia[:, :],
            )
            return fT

        fT_v = side_head(SP_VW1, vb1_t, vfT_sb, BI, "v")
        fT_u = side_head(SP_UW1, ub1_t, ufT_sb, BU, "u")

        # ---- pass 2 (fully local): out = relu(fac*relu(H)@dW^T + F@W2^T) ----
        def pass2(h_red, fT, w2c, fac, n, o_dram, nm):
            hT = []
            for mh in range(2):
                hf = scr.tile([128, n], BF, tag="p2h", bufs=4, name="hf")
                nc.sync.dma_start(
                    out=hf[:, :], in_=h_red[mh * 128 : (mh + 1) * 128, :]
                )
                hb = scr.tile([128, n], BF, tag="p2hb", bufs=4, name="hb")
                nc.scalar.activation(out=hb[:, :], in_=hf[:, :], func=AF.Relu)
                hT.append(hb)
            for kt, (s, p) in enumerate(PT):
                pa = ps_p2.tile([128, OUT], FP, tag="pa", name="pa")
                for mh in range(2):
                    nc.tensor.matmul(
                        pa[:p, :], lhsT=hT[mh][:, s : s + p],
                        rhs=sp_sb[:128, SP_DW + mh * OUT : SP_DW + (mh + 1) * OUT],
                        start=(mh == 0), stop=(mh == 1),
                    )
                sa = scr.tile([128, OUT], FP, tag="p2sa", name="sa")
                nc.scalar.activation(
                    out=sa[:p, :], in_=pa[:p, :], func=AF.Copy, scale=fac[kt][:p, :]
                )
                pb = ps_p2.tile([128, OUT], FP, tag="pb", name="pb")
                nc.tensor.matmul(
                    pb[:p, :], lhsT=fT[:SIDE, s : s + p],
                    rhs=sp_sb[:SIDE, w2c : w2c + OUT],
                    start=True, stop=True,
                )
                so = scr.tile([128, OUT], FP, tag="p2so", name="so")
                nc.vector.tensor_tensor(
                    out=so[:p, :], in0=pb[:p, :], in1=sa[:p, :], op=ALU.add
                )
                ro = scr.tile([128, OUT], FP, tag="p2ro", name="ro")
                nc.scalar.activation(out=ro[:p, :], in_=so[:p, :], func=AF.Relu)
                nc.sync.dma_start(out=o_dram[s : s + p, :], in_=ro[:p, :])

        pass2(dram_hv_red, fT_v, SP_VW2, b_fac, BI, v_out, "v")
        pass2(dram_hu_red, fT_u, SP_UW2, a_fac, BU, u_out, "u")

    nc.compile()
    return nc


_CACHE = {}


def _get_program():
    if "nc" not in _CACHE:
        _CACHE["nc"] = build_program()
    return _CACHE["nc"]


def _pack_w(w_slice):
    # w_slice: [R, M, 500] bf16 -> [4, 125, R*M] with chunk kt, col (r*M + m)
    return np.ascontiguousarray(
        w_slice.reshape(R, M, 4, 125).transpose(2, 3, 0, 1).reshape(4, 125, R * M)
    )


def make_in_maps(inputs):
    import ml_dtypes

    bf = ml_dtypes.bfloat16
    adj = np.asarray(inputs["adj_matrix"], dtype=np.int32)
    adjB = adj.astype(np.int8)  # values 0..5
    msg_W = np.asarray(inputs["msg_W"], np.float32).astype(bf)
    u_sfT = np.asarray(inputs["u_sideFeat"], np.float32).astype(bf).T
    v_sfT = np.asarray(inputs["v_sideFeat"], np.float32).astype(bf).T
    ub1 = np.asarray(inputs["u_b1"], np.float32).reshape(SIDE, 1)
    vb1 = np.asarray(inputs["v_b1"], np.float32).reshape(SIDE, 1)

    sp = np.zeros((128, SP_COLS), bf)
    dw = np.asarray(inputs["dense_W"], np.float32).astype(bf)  # [75, 256]
    sp[:, SP_DW : SP_DW + 150] = dw.T.reshape(2, 128, OUT).transpose(1, 0, 2).reshape(
        128, 150
    )
    sp[:, SP_UW1 : SP_UW1 + SIDE] = np.asarray(inputs["u_W1"], np.float32).astype(bf).T
    sp[:, SP_VW1 : SP_VW1 + SIDE] = np.asarray(inputs["v_W1"], np.float32).astype(bf).T
    sp[:SIDE, SP_UW2 : SP_UW2 + OUT] = (
        np.asarray(inputs["u_W2"], np.float32).astype(bf).T
    )
    sp[:SIDE, SP_VW2 : SP_VW2 + OUT] = (
        np.asarray(inputs["v_W2"], np.float32).astype(bf).T
    )

    in_maps = []
    for c in range(NCORES):
        us, ie = c * BU, c * BI
        in_maps.append(
            {
                "adjR": np.ascontiguousarray(adjB[us : us + BU, :]),
                "adjCT": np.ascontiguousarray(adjB[:, ie : ie + BI].T),
                "wuH": _pack_w(msg_W[:, :, us : us + BU]),
                "wiH": _pack_w(msg_W[:, :, NU + ie : NU + ie + BI]),
                "ufT": np.ascontiguousarray(u_sfT[:, us : us + BU]),
                "vfT": np.ascontiguousarray(v_sfT[:, ie : ie + BI]),
                "smallpack": sp,
                "ub1": ub1,
                "vb1": vb1,
            }
        )
    return in_maps


def assemble(results):
    U = np.empty((NU, OUT), np.float32)
    V = np.empty((NI, OUT), np.float32)
    for c in range(NCORES):
        U[c * BU : (c + 1) * BU] = results[c]["u_out"]
        V[c * BI : (c + 1) * BI] = results[c]["v_out"]
    return (U, V)


def kernel(**inputs):
    from concourse.bass_utils import run_bass_kernel_spmd

    nc = _get_program()
    res = run_bass_kernel_spmd(nc, make_in_maps(inputs), core_ids=list(range(NCORES)))
    return assemble(res.results)
